# revision 2
# baseline (speedup 1.0000x reference)
"""Neural ODE (RK4 over a 2->512->512->256->2 tanh MLP) on 8 Trainium2 cores.

Strategy: data-parallel over the batch (65536 rows -> 8192/core), with a
feature-on-partition layout so the MLP weights are PE-stationary and the
batch streams through as the matmul free dimension.  Each core runs the
99 sequential RK4 steps in a single-hardware-loop (For_i) body covering
2 interleaved batch "streams" of 2048 columns (so TensorE can work on one
stream while ScalarE runs tanh on the other), repeated for 2 phases to
cover all 4 x 2048 = 8192 rows.

Numerics: fp16 matmul inputs (weights + activations), fp32 PSUM
accumulation, fp32 state/trajectory.  dt is uniform (t_points = arange*0.01)
and is baked into the instruction stream as immediates.
"""

import numpy as np

import concourse.bass as bass
import concourse.mybir as mybir
import concourse.tile as tile
from concourse import bacc
from concourse.bass import ds
from concourse.bass_utils import run_bass_kernel_spmd

FP32 = mybir.dt.float32
FP16 = mybir.dt.float16
AF = mybir.ActivationFunctionType
ALU = mybir.AluOpType

NCORES = 8
B = 65536
BC = B // NCORES          # 8192 rows per core
T = 100
S = T - 1                 # 99 RK4 steps
H = 512
NMACRO = 2048             # columns per stream
NMACROS = BC // NMACRO    # 4 macros per core
NSTREAMS = 2
NPHASES = NMACROS // NSTREAMS


def _build_program(dt: float, b4_nonzero: bool):
    nc = bacc.Bacc("TRN2", num_devices=NCORES)

    y0t = nc.declare_dram_parameter("y0t", [2, BC], FP32, isOutput=False)
    w1 = nc.declare_dram_parameter("w1", [2, H], FP16, isOutput=False)
    w2 = nc.declare_dram_parameter("w2", [128, 2048], FP16, isOutput=False)
    w3 = nc.declare_dram_parameter("w3", [128, 1024], FP16, isOutput=False)
    w4 = nc.declare_dram_parameter("w4", [128, 4], FP16, isOutput=False)
    b1 = nc.declare_dram_parameter("b1", [128, 4], FP32, isOutput=False)
    b2 = nc.declare_dram_parameter("b2", [128, 4], FP32, isOutput=False)
    b3 = nc.declare_dram_parameter("b3", [128, 2], FP32, isOutput=False)
    b4 = nc.declare_dram_parameter("b4", [2, 1], FP32, isOutput=False)
    b4c = nc.declare_dram_parameter("b4c", [2, 1], FP32, isOutput=False)
    outp = nc.declare_dram_parameter("out", [NMACROS, S, 2, NMACRO], FP32, isOutput=True)

    dt2, dtf, dt6 = float(dt) * 0.5, float(dt), float(dt) / 6.0

    with tile.TileContext(nc) as tc:
        with (
            tc.tile_pool(name="wp", bufs=1) as wp,
            tc.tile_pool(name="state", bufs=1) as state,
            tc.tile_pool(name="hp", bufs=1) as hp,
            tc.tile_pool(name="y16p", bufs=2) as y16p,
            tc.tile_pool(name="psA", bufs=2, space="PSUM") as psA,
            tc.tile_pool(name="psB", bufs=2, space="PSUM") as psB,
        ):
            ps = [psA, psB]

            w1t = wp.tile([2, H], FP16, tag="w1")
            w2t = wp.tile([128, 2048], FP16, tag="w2")
            w3t = wp.tile([128, 1024], FP16, tag="w3")
            w4t = wp.tile([128, 4], FP16, tag="w4")
            b1t = wp.tile([128, 4], FP32, tag="b1")
            b2t = wp.tile([128, 4], FP32, tag="b2")
            b3t = wp.tile([128, 2], FP32, tag="b3")
            b4t = wp.tile([2, 1], FP32, tag="b4")
            b4ct = wp.tile([2, 1], FP32, tag="b4c")
            for t_, src in ((w1t, w1), (w2t, w2), (w3t, w3), (w4t, w4),
                            (b1t, b1), (b2t, b2), (b3t, b3), (b4t, b4), (b4ct, b4c)):
                nc.sync.dma_start(out=t_[:], in_=src[:])

            # persistent per-stream fp32 state + k buffers
            y = [state.tile([2, NMACRO], FP32, tag=f"y{s}", name=f"y{s}") for s in range(NSTREAMS)]
            ks = [[state.tile([2, NMACRO], FP32, tag=f"k{s}_{e}", name=f"k{s}_{e}") for e in range(3)]
                  for s in range(NSTREAMS)]
            sacc = [state.tile([2, NMACRO], FP32, tag=f"s{s}", name=f"s{s}") for s in range(NSTREAMS)]

            def mlp_eval(st, rhs16, h1, h2, h3):
                """Emit one MLP eval for stream st; returns the two [2,1024] L4 psums."""
                # L1: 2 -> 512
                for m in range(4):
                    for hf in range(2):
                        p = ps[st].tile([128, 1024], FP32, tag=f"p{st}", name=f"p{st}")
                        for n in range(2):
                            nc.tensor.matmul(
                                p[:, 512 * n:512 * (n + 1)],
                                w1t[:, 128 * m:128 * (m + 1)],
                                rhs16[:, 1024 * hf + 512 * n:1024 * hf + 512 * (n + 1)],
                                start=True, stop=True)
                        nc.scalar.activation(
                            h1[:, 2048 * m + 1024 * hf:2048 * m + 1024 * (hf + 1)],
                            p[:], AF.Tanh, bias=b1t[:, m:m + 1])
                # L2: 512 -> 512
                for m2 in range(4):
                    for hf in range(2):
                        p = ps[st].tile([128, 1024], FP32, tag=f"p{st}", name=f"p{st}")
                        for k in range(4):
                            for n in range(2):
                                nc.tensor.matmul(
                                    p[:, 512 * n:512 * (n + 1)],
                                    w2t[:, (k * 4 + m2) * 128:(k * 4 + m2 + 1) * 128],
                                    h1[:, 2048 * k + 1024 * hf + 512 * n:
                                       2048 * k + 1024 * hf + 512 * (n + 1)],
                                    start=(k == 0), stop=(k == 3))
                        nc.scalar.activation(
                            h2[:, 2048 * m2 + 1024 * hf:2048 * m2 + 1024 * (hf + 1)],
                            p[:], AF.Tanh, bias=b2t[:, m2:m2 + 1])
                # L3: 512 -> 256
                for m3 in range(2):
                    for hf in range(2):
                        p = ps[st].tile([128, 1024], FP32, tag=f"p{st}", name=f"p{st}")
                        for k in range(4):
                            for n in range(2):
                                nc.tensor.matmul(
                                    p[:, 512 * n:512 * (n + 1)],
                                    w3t[:, (k * 2 + m3) * 128:(k * 2 + m3 + 1) * 128],
                                    h2[:, 2048 * k + 1024 * hf + 512 * n:
                                       2048 * k + 1024 * hf + 512 * (n + 1)],
                                    start=(k == 0), stop=(k == 3))
                        nc.scalar.activation(
                            h3[:, 2048 * m3 + 1024 * hf:2048 * m3 + 1024 * (hf + 1)],
                            p[:], AF.Tanh, bias=b3t[:, m3:m3 + 1])
                # L4: 256 -> 2 (raw psum, bias handled downstream)
                p4 = []
                for hf in range(2):
                    p = ps[st].tile([2, 1024], FP32, tag=f"p{st}", name=f"p4_{st}")
                    for k in range(2):
                        for n in range(2):
                            nc.tensor.matmul(
                                p[:, 512 * n:512 * (n + 1)],
                                w4t[:, 2 * k:2 * k + 2],
                                h3[:, 2048 * k + 1024 * hf + 512 * n:
                                   2048 * k + 1024 * hf + 512 * (n + 1)],
                                start=(k == 0), stop=(k == 1))
                    p4.append(p)
                return p4

            for phase in range(NPHASES):
                macros = [phase * NSTREAMS + s for s in range(NSTREAMS)]
                for st in range(NSTREAMS):
                    m0 = macros[st] * NMACRO
                    nc.sync.dma_start(out=y[st][:], in_=y0t[:, m0:m0 + NMACRO])

                with tc.For_i(0, S, 1, hint_engines=(mybir.EngineType.PE,)) as i:
                    h1 = [hp.tile([128, 4 * 2048], FP16, tag=f"h1_{s}", name=f"h1_{s}") for s in range(NSTREAMS)]
                    h2 = [hp.tile([128, 4 * 2048], FP16, tag=f"h2_{s}", name=f"h2_{s}") for s in range(NSTREAMS)]
                    h3 = [hp.tile([128, 2 * 2048], FP16, tag=f"h3_{s}", name=f"h3_{s}") for s in range(NSTREAMS)]

                    rhs = []
                    for st in range(NSTREAMS):
                        y16 = y16p.tile([2, NMACRO], FP16, tag=f"y16_{st}", name=f"y16_{st}")
                        nc.vector.tensor_copy(y16[:], y[st][:])
                        rhs.append(y16)

                    for e in range(4):
                        p4s = [mlp_eval(st, rhs[st], h1[st], h2[st], h3[st])
                               for st in range(NSTREAMS)]
                        new_rhs = []
                        for st in range(NSTREAMS):
                            p4 = p4s[st]
                            if e < 3:
                                ke = ks[st][e]
                                for hf in range(2):
                                    nc.vector.tensor_scalar(
                                        ke[:, 1024 * hf:1024 * (hf + 1)], p4[hf][:],
                                        b4t[:, 0:1], None, ALU.add)
                                c = dt2 if e < 2 else dtf
                                ynext16 = y16p.tile([2, NMACRO], FP16, tag=f"y16_{st}", name=f"y16_{st}")
                                nc.vector.scalar_tensor_tensor(
                                    ynext16[:], ke[:], c, y[st][:], ALU.mult, ALU.add)
                                new_rhs.append(ynext16)
                            else:
                                # y += dt/6 * (k1 + 2*(k2+k3) + p4(+b4))
                                sa = sacc[st]
                                nc.vector.tensor_tensor(sa[:], ks[st][1][:], ks[st][2][:], ALU.add)
                                nc.vector.scalar_tensor_tensor(
                                    sa[:], sa[:], 2.0, ks[st][0][:], ALU.mult, ALU.add)
                                for hf in range(2):
                                    nc.vector.tensor_tensor(
                                        sa[:, 1024 * hf:1024 * (hf + 1)],
                                        sa[:, 1024 * hf:1024 * (hf + 1)],
                                        p4[hf][:], ALU.add)
                                nc.vector.scalar_tensor_tensor(
                                    y[st][:], sa[:], dt6, y[st][:], ALU.mult, ALU.add)
                                if b4_nonzero:
                                    nc.vector.tensor_scalar(
                                        y[st][:], y[st][:], b4ct[:, 0:1], None, ALU.add)
                                nc.sync.dma_start(
                                    out=outp[macros[st]][ds(i, 1), :, :], in_=y[st][:])
                        rhs = new_rhs

    nc.compile()
    return nc


def kernel(y0, t_points, W1, b1, W2, b2, W3, b3, W4, b4):
    y0 = np.asarray(y0, dtype=np.float32)
    t_points = np.asarray(t_points, dtype=np.float32)
    W1 = np.asarray(W1, dtype=np.float32)
    W2 = np.asarray(W2, dtype=np.float32)
    W3 = np.asarray(W3, dtype=np.float32)
    W4 = np.asarray(W4, dtype=np.float32)
    b1 = np.asarray(b1, dtype=np.float32)
    b2 = np.asarray(b2, dtype=np.float32)
    b3 = np.asarray(b3, dtype=np.float32)
    b4 = np.asarray(b4, dtype=np.float32)

    dts = (t_points[1:] - t_points[:-1]).astype(np.float64)
    dt = float(np.mean(dts))
    b4_nonzero = bool(np.any(b4 != 0.0))

    nc = _build_program(dt, b4_nonzero)

    # host-side packing into PE-friendly layouts
    w1p = W1.astype(np.float16)                                   # [2, 512]
    w2p = W2.reshape(4, 128, 4, 128).transpose(1, 0, 2, 3).reshape(128, 2048).astype(np.float16)
    w3p = W3.reshape(4, 128, 2, 128).transpose(1, 0, 2, 3).reshape(128, 1024).astype(np.float16)
    w4p = W4.reshape(2, 128, 2).transpose(1, 0, 2).reshape(128, 4).astype(np.float16)
    b1p = np.ascontiguousarray(b1.reshape(4, 128).T)              # [128, 4]
    b2p = np.ascontiguousarray(b2.reshape(4, 128).T)
    b3p = np.ascontiguousarray(b3.reshape(2, 128).T)
    b4p = np.ascontiguousarray(b4.reshape(2, 1))
    b4cp = np.ascontiguousarray((b4 * (dt / 6.0)).astype(np.float32).reshape(2, 1))

    in_maps = []
    for c in range(NCORES):
        y0c = np.ascontiguousarray(y0[c * BC:(c + 1) * BC, :].T)  # [2, 8192]
        in_maps.append({
            "y0t": y0c, "w1": w1p, "w2": w2p, "w3": w3p, "w4": w4p,
            "b1": b1p, "b2": b2p, "b3": b3p, "b4": b4p, "b4c": b4cp,
        })

    res = run_bass_kernel_spmd(nc, in_maps, list(range(NCORES)))

    full = np.empty((T, B, 2), dtype=np.float32)
    full[0] = y0
    for c in range(NCORES):
        oc = res.results[c]["out"]                                # [4, 99, 2, 2048]
        for m in range(NMACROS):
            col0 = c * BC + m * NMACRO
            full[1:, col0:col0 + NMACRO, :] = oc[m].transpose(0, 2, 1)
    return full


# revision 3
# speedup vs baseline: 1.0546x; 1.0546x over previous
"""Neural ODE (RK4 over a 2->512->512->256->2 tanh MLP) on 8 Trainium2 cores.

Strategy: data-parallel over the batch (65536 rows -> 8192/core), with a
feature-on-partition layout so the MLP weights are PE-stationary and the
batch streams through as the matmul free dimension.  Each core runs the
99 sequential RK4 steps in a single-hardware-loop (For_i) body covering
2 interleaved batch "streams" of 2048 columns (so TensorE can work on one
stream while ScalarE runs tanh on the other), repeated for 2 phases to
cover all 4 x 2048 = 8192 rows.

Numerics: fp16 matmul inputs (weights + activations), fp32 PSUM
accumulation, fp32 state/trajectory.  dt is uniform (t_points = arange*0.01)
and is baked into the instruction stream as immediates.

Critical-path notes: the intermediate RK4 states y+c*k are formed by a
single fused DVE op directly from the layer-4 PSUM (the b4 bias term is
folded into the next eval's layer-1 activation bias), so TensorE only
waits ~2us at each eval boundary; k backups for the final combine are
copied out of PSUM off the critical path, and the combine partial sums
run during eval 4.
"""

import numpy as np

import concourse.bass as bass
import concourse.mybir as mybir
import concourse.tile as tile
from concourse import bacc
from concourse.bass import ds
from concourse.bass_utils import run_bass_kernel_spmd

FP32 = mybir.dt.float32
FP16 = mybir.dt.float16
AF = mybir.ActivationFunctionType
ALU = mybir.AluOpType

NCORES = 8
B = 65536
BC = B // NCORES          # 8192 rows per core
T = 100
S = T - 1                 # 99 RK4 steps
H = 512
NMACRO = 2048             # columns per stream
NMACROS = BC // NMACRO    # 4 macros per core
NSTREAMS = 2
NPHASES = NMACROS // NSTREAMS


def _build_program(dt: float, b4_nonzero: bool):
    nc = bacc.Bacc("TRN2", num_devices=NCORES)

    y0t = nc.declare_dram_parameter("y0t", [2, BC], FP32, isOutput=False)
    w1 = nc.declare_dram_parameter("w1", [2, H], FP16, isOutput=False)
    w2 = nc.declare_dram_parameter("w2", [128, 2048], FP16, isOutput=False)
    w3 = nc.declare_dram_parameter("w3", [128, 1024], FP16, isOutput=False)
    w4 = nc.declare_dram_parameter("w4", [128, 4], FP16, isOutput=False)
    # per-eval layer-1 biases (b4 folded in: b1_e = b1 + c_e * (b4 @ W1))
    b1e = nc.declare_dram_parameter("b1e", [128, 12], FP32, isOutput=False)
    b2 = nc.declare_dram_parameter("b2", [128, 4], FP32, isOutput=False)
    b3 = nc.declare_dram_parameter("b3", [128, 2], FP32, isOutput=False)
    b4c = nc.declare_dram_parameter("b4c", [2, 1], FP32, isOutput=False)  # dt*b4
    outp = nc.declare_dram_parameter("out", [NMACROS, S, 2, NMACRO], FP32, isOutput=True)

    dt2, dtf, dt6 = float(dt) * 0.5, float(dt), float(dt) / 6.0

    with tile.TileContext(nc) as tc:
        with (
            tc.tile_pool(name="wp", bufs=1) as wp,
            tc.tile_pool(name="state", bufs=1) as state,
            tc.tile_pool(name="hp", bufs=1) as hp,
            tc.tile_pool(name="psA", bufs=2, space="PSUM") as psA,
            tc.tile_pool(name="psB", bufs=2, space="PSUM") as psB,
        ):
            ps = [psA, psB]

            w1t = wp.tile([2, H], FP16, tag="w1")
            w2t = wp.tile([128, 2048], FP16, tag="w2")
            w3t = wp.tile([128, 1024], FP16, tag="w3")
            w4t = wp.tile([128, 4], FP16, tag="w4")
            b1et = wp.tile([128, 12], FP32, tag="b1e")
            b2t = wp.tile([128, 4], FP32, tag="b2")
            b3t = wp.tile([128, 2], FP32, tag="b3")
            b4ct = wp.tile([2, 1], FP32, tag="b4c")
            for t_, src in ((w1t, w1), (w2t, w2), (w3t, w3), (w4t, w4),
                            (b1et, b1e), (b2t, b2), (b3t, b3), (b4ct, b4c)):
                nc.sync.dma_start(out=t_[:], in_=src[:])
            # b1 column for eval e (0-based): evals 0..3 -> cols 0,1,1,2 blocks of 4
            b1col = [0, 4, 4, 8]

            # persistent per-stream fp32 state, k backups, combine acc, fp16 rhs
            y = [state.tile([2, NMACRO], FP32, tag=f"y{s}", name=f"y{s}")
                 for s in range(NSTREAMS)]
            ks = [[state.tile([2, NMACRO], FP32, tag=f"k{s}_{e}", name=f"k{s}_{e}")
                   for e in range(3)] for s in range(NSTREAMS)]
            sacc = [state.tile([2, NMACRO], FP32, tag=f"s{s}", name=f"s{s}")
                    for s in range(NSTREAMS)]
            y16 = [state.tile([2, NMACRO], FP16, tag=f"y16_{s}", name=f"y16_{s}")
                   for s in range(NSTREAMS)]
            yin = [[state.tile([2, NMACRO], FP16, tag=f"yin{s}_{e}", name=f"yin{s}_{e}")
                    for e in range(3)] for s in range(NSTREAMS)]

            def mlp_eval(st, rhs16, h1, h2, h3, e):
                """Emit one MLP eval for stream st; returns the two [2,1024] L4 psums."""
                # L1: 2 -> 512
                for m in range(4):
                    for hf in range(2):
                        p = ps[st].tile([128, 1024], FP32, tag=f"p{st}", name=f"p{st}")
                        for n in range(2):
                            nc.tensor.matmul(
                                p[:, 512 * n:512 * (n + 1)],
                                w1t[:, 128 * m:128 * (m + 1)],
                                rhs16[:, 1024 * hf + 512 * n:1024 * hf + 512 * (n + 1)],
                                start=True, stop=True)
                        nc.scalar.activation(
                            h1[:, 2048 * m + 1024 * hf:2048 * m + 1024 * (hf + 1)],
                            p[:], AF.Tanh, bias=b1et[:, b1col[e] + m:b1col[e] + m + 1])
                # L2: 512 -> 512
                for m2 in range(4):
                    for hf in range(2):
                        p = ps[st].tile([128, 1024], FP32, tag=f"p{st}", name=f"p{st}")
                        for k in range(4):
                            for n in range(2):
                                nc.tensor.matmul(
                                    p[:, 512 * n:512 * (n + 1)],
                                    w2t[:, (k * 4 + m2) * 128:(k * 4 + m2 + 1) * 128],
                                    h1[:, 2048 * k + 1024 * hf + 512 * n:
                                       2048 * k + 1024 * hf + 512 * (n + 1)],
                                    start=(k == 0), stop=(k == 3))
                        nc.scalar.activation(
                            h2[:, 2048 * m2 + 1024 * hf:2048 * m2 + 1024 * (hf + 1)],
                            p[:], AF.Tanh, bias=b2t[:, m2:m2 + 1])
                # L3: 512 -> 256
                for m3 in range(2):
                    for hf in range(2):
                        p = ps[st].tile([128, 1024], FP32, tag=f"p{st}", name=f"p{st}")
                        for k in range(4):
                            for n in range(2):
                                nc.tensor.matmul(
                                    p[:, 512 * n:512 * (n + 1)],
                                    w3t[:, (k * 2 + m3) * 128:(k * 2 + m3 + 1) * 128],
                                    h2[:, 2048 * k + 1024 * hf + 512 * n:
                                       2048 * k + 1024 * hf + 512 * (n + 1)],
                                    start=(k == 0), stop=(k == 3))
                        nc.scalar.activation(
                            h3[:, 2048 * m3 + 1024 * hf:2048 * m3 + 1024 * (hf + 1)],
                            p[:], AF.Tanh, bias=b3t[:, m3:m3 + 1])
                # L4: 256 -> 2 (raw psum, p_e = k_e - b4; b4 handled via bias folds)
                p4 = []
                for hf in range(2):
                    p = ps[st].tile([2, 1024], FP32, tag=f"p{st}", name=f"p4_{st}")
                    for k in range(2):
                        for n in range(2):
                            nc.tensor.matmul(
                                p[:, 512 * n:512 * (n + 1)],
                                w4t[:, 2 * k:2 * k + 2],
                                h3[:, 2048 * k + 1024 * hf + 512 * n:
                                   2048 * k + 1024 * hf + 512 * (n + 1)],
                                start=(k == 0), stop=(k == 1))
                    p4.append(p)
                return p4

            def post_eval(st, e, p4, i):
                """DVE work after eval e of stream st (p4 = raw L4 psum halves)."""
                cs = (dt2, dt2, dtf)
                if e < 3:
                    # critical path: next eval's input, straight from PSUM
                    nxt = yin[st][e]
                    for hf in range(2):
                        sl = slice(1024 * hf, 1024 * (hf + 1))
                        nc.vector.scalar_tensor_tensor(
                            nxt[:, sl], p4[hf][:], cs[e], y[st][:, sl],
                            ALU.mult, ALU.add)
                    # off critical path: back up p_e for the final combine
                    ke = ks[st][e]
                    for hf in range(2):
                        nc.vector.tensor_copy(
                            ke[:, 1024 * hf:1024 * (hf + 1)], p4[hf][:])
                    if e == 2:
                        # combine partials run during eval 4's matmuls:
                        # sacc = k1 + 2*(k2 + k3)
                        sa = sacc[st]
                        nc.vector.tensor_tensor(sa[:], ks[st][1][:], ks[st][2][:], ALU.add)
                        nc.vector.scalar_tensor_tensor(
                            sa[:], sa[:], 2.0, ks[st][0][:], ALU.mult, ALU.add)
                    return nxt
                # e == 3: y += dt/6 * (sacc + p4)   (+ dt*b4 fold if nonzero)
                sa = sacc[st]
                for hf in range(2):
                    sl = slice(1024 * hf, 1024 * (hf + 1))
                    nc.vector.tensor_tensor(sa[:, sl], sa[:, sl], p4[hf][:], ALU.add)
                nc.vector.scalar_tensor_tensor(
                    y[st][:], sa[:], dt6, y[st][:], ALU.mult, ALU.add)
                if b4_nonzero:
                    nc.vector.tensor_scalar(
                        y[st][:], y[st][:], b4ct[:, 0:1], None, ALU.add)
                # fp16 rhs for the next step's eval 1
                nc.vector.tensor_copy(y16[st][:], y[st][:])
                return None

            for phase in range(NPHASES):
                macros = [phase * NSTREAMS + s for s in range(NSTREAMS)]
                for st in range(NSTREAMS):
                    m0 = macros[st] * NMACRO
                    nc.sync.dma_start(out=y[st][:], in_=y0t[:, m0:m0 + NMACRO])
                    nc.vector.tensor_copy(y16[st][:], y[st][:])

                with tc.For_i(0, S, 1, hint_engines=(mybir.EngineType.PE,)) as i:
                    h1 = [hp.tile([128, 4 * 2048], FP16, tag=f"h1_{s}", name=f"h1_{s}")
                          for s in range(NSTREAMS)]
                    h2 = [hp.tile([128, 4 * 2048], FP16, tag=f"h2_{s}", name=f"h2_{s}")
                          for s in range(NSTREAMS)]
                    h3 = [hp.tile([128, 2 * 2048], FP16, tag=f"h3_{s}", name=f"h3_{s}")
                          for s in range(NSTREAMS)]

                    rhs = [y16[st] for st in range(NSTREAMS)]
                    for e in range(4):
                        new_rhs = []
                        for st in range(NSTREAMS):
                            p4 = mlp_eval(st, rhs[st], h1[st], h2[st], h3[st], e)
                            new_rhs.append(post_eval(st, e, p4, i))
                        rhs = new_rhs
                    for st in range(NSTREAMS):
                        nc.sync.dma_start(
                            out=outp[macros[st]][ds(i, 1), :, :], in_=y[st][:])

    nc.compile()
    return nc


def kernel(y0, t_points, W1, b1, W2, b2, W3, b3, W4, b4):
    y0 = np.asarray(y0, dtype=np.float32)
    t_points = np.asarray(t_points, dtype=np.float32)
    W1 = np.asarray(W1, dtype=np.float32)
    W2 = np.asarray(W2, dtype=np.float32)
    W3 = np.asarray(W3, dtype=np.float32)
    W4 = np.asarray(W4, dtype=np.float32)
    b1 = np.asarray(b1, dtype=np.float32)
    b2 = np.asarray(b2, dtype=np.float32)
    b3 = np.asarray(b3, dtype=np.float32)
    b4 = np.asarray(b4, dtype=np.float32)

    dts = (t_points[1:] - t_points[:-1]).astype(np.float64)
    dt = float(np.mean(dts))
    b4_nonzero = bool(np.any(b4 != 0.0))

    nc = _build_program(dt, b4_nonzero)

    # host-side packing into PE-friendly layouts
    w1p = W1.astype(np.float16)                                   # [2, 512]
    w2p = W2.reshape(4, 128, 4, 128).transpose(1, 0, 2, 3).reshape(128, 2048).astype(np.float16)
    w3p = W3.reshape(4, 128, 2, 128).transpose(1, 0, 2, 3).reshape(128, 1024).astype(np.float16)
    w4p = W4.reshape(2, 128, 2).transpose(1, 0, 2).reshape(128, 4).astype(np.float16)

    bb = (b4.astype(np.float64) @ W1.astype(np.float64))          # [512]
    b1_sets = [b1, (b1 + (dt / 2) * bb).astype(np.float32),
               (b1 + dt * bb).astype(np.float32)]
    b1ep = np.concatenate([np.ascontiguousarray(v.reshape(4, 128).T)
                           for v in b1_sets], axis=1)             # [128, 12]
    b1ep = np.ascontiguousarray(b1ep, dtype=np.float32)
    b2p = np.ascontiguousarray(b2.reshape(4, 128).T)
    b3p = np.ascontiguousarray(b3.reshape(2, 128).T)
    b4cp = np.ascontiguousarray((b4 * dt).astype(np.float32).reshape(2, 1))

    in_maps = []
    for c in range(NCORES):
        y0c = np.ascontiguousarray(y0[c * BC:(c + 1) * BC, :].T)  # [2, 8192]
        in_maps.append({
            "y0t": y0c, "w1": w1p, "w2": w2p, "w3": w3p, "w4": w4p,
            "b1e": b1ep, "b2": b2p, "b3": b3p, "b4c": b4cp,
        })

    res = run_bass_kernel_spmd(nc, in_maps, list(range(NCORES)))

    full = np.empty((T, B, 2), dtype=np.float32)
    full[0] = y0
    for c in range(NCORES):
        oc = res.results[c]["out"]                                # [4, 99, 2, 2048]
        for m in range(NMACROS):
            col0 = c * BC + m * NMACRO
            full[1:, col0:col0 + NMACRO, :] = oc[m].transpose(0, 2, 1)
    return full


# revision 4
# speedup vs baseline: 1.1401x; 1.0811x over previous
"""Neural ODE (RK4 over a 2->512->512->256->2 tanh MLP) on 8 Trainium2 cores.

Strategy: data-parallel over the batch (65536 rows -> 8192/core), with a
feature-on-partition layout so the MLP weights are PE-stationary and the
batch streams through as the matmul free dimension.  Each core runs the
99 sequential RK4 steps in a hardware loop (For_i, 2 steps per body) over
4 independent batch "streams" of 1024 columns, so TensorE always has
another stream's matmuls to run while one stream waits on tanh (ScalarE)
or the RK4 state update (VectorE); 2 phases cover all 8192 rows.

Numerics: fp16 matmul inputs (weights + activations), fp32 PSUM
accumulation, fp32 state/trajectory.  dt is uniform (t_points = arange*0.01)
and is baked into the instruction stream as immediates.

Critical-path notes: the intermediate RK4 states y+c*k are formed by a
single fused DVE op directly from the layer-4 PSUM (the b4 bias term is
folded into the next eval's layer-1 activation bias), k backups for the
final combine are copied out of PSUM off the critical path, and the
combine partial sums run during eval 4's matmuls.
"""

import numpy as np

import concourse.bass as bass
import concourse.mybir as mybir
import concourse.tile as tile
from concourse import bacc
from concourse.bass import ds
from concourse.bass_utils import run_bass_kernel_spmd

FP32 = mybir.dt.float32
FP16 = mybir.dt.float16
AF = mybir.ActivationFunctionType
ALU = mybir.AluOpType

NCORES = 8
B = 65536
BC = B // NCORES          # 8192 rows per core
T = 100
S = T - 1                 # 99 RK4 steps
H = 512
NS = 1024                 # columns per stream
NSTREAMS = 4
NCHUNK = BC // NS         # 8 chunks of 1024 per core
NPHASES = NCHUNK // NSTREAMS


def _build_program(dt: float, b4_nonzero: bool):
    nc = bacc.Bacc("TRN2", num_devices=NCORES)

    y0t = nc.declare_dram_parameter("y0t", [2, BC], FP32, isOutput=False)
    w1 = nc.declare_dram_parameter("w1", [2, H], FP16, isOutput=False)
    w2 = nc.declare_dram_parameter("w2", [128, 2048], FP16, isOutput=False)
    w3 = nc.declare_dram_parameter("w3", [128, 1024], FP16, isOutput=False)
    w4 = nc.declare_dram_parameter("w4", [128, 4], FP16, isOutput=False)
    # per-eval layer-1 biases (b4 folded in: b1_e = b1 + c_e * (b4 @ W1))
    b1e = nc.declare_dram_parameter("b1e", [128, 12], FP32, isOutput=False)
    b2 = nc.declare_dram_parameter("b2", [128, 4], FP32, isOutput=False)
    b3 = nc.declare_dram_parameter("b3", [128, 2], FP32, isOutput=False)
    b4c = nc.declare_dram_parameter("b4c", [2, 1], FP32, isOutput=False)  # dt*b4
    outp = nc.declare_dram_parameter("out", [NCHUNK, S, 2, NS], FP32, isOutput=True)

    dt2, dtf, dt6 = float(dt) * 0.5, float(dt), float(dt) / 6.0
    b1col = [0, 4, 4, 8]

    with tile.TileContext(nc) as tc:
        with (
            tc.tile_pool(name="wp", bufs=1) as wp,
            tc.tile_pool(name="state", bufs=1) as state,
            tc.tile_pool(name="hp", bufs=1) as hp,
            tc.tile_pool(name="ps0", bufs=1, space="PSUM") as ps0,
            tc.tile_pool(name="ps1", bufs=1, space="PSUM") as ps1,
            tc.tile_pool(name="ps2", bufs=1, space="PSUM") as ps2,
            tc.tile_pool(name="ps3", bufs=1, space="PSUM") as ps3,
        ):
            ps = [ps0, ps1, ps2, ps3]

            w1t = wp.tile([2, H], FP16, tag="w1")
            w2t = wp.tile([128, 2048], FP16, tag="w2")
            w3t = wp.tile([128, 1024], FP16, tag="w3")
            w4t = wp.tile([128, 4], FP16, tag="w4")
            b1et = wp.tile([128, 12], FP32, tag="b1e")
            b2t = wp.tile([128, 4], FP32, tag="b2")
            b3t = wp.tile([128, 2], FP32, tag="b3")
            b4ct = wp.tile([2, 1], FP32, tag="b4c")
            for t_, src in ((w1t, w1), (w2t, w2), (w3t, w3), (w4t, w4),
                            (b1et, b1e), (b2t, b2), (b3t, b3), (b4ct, b4c)):
                nc.sync.dma_start(out=t_[:], in_=src[:])

            # persistent per-stream fp32 state, k backups, combine acc, fp16 rhs
            y = [state.tile([2, NS], FP32, tag=f"y{s}", name=f"y{s}")
                 for s in range(NSTREAMS)]
            ks = [[state.tile([2, NS], FP32, tag=f"k{s}_{e}", name=f"k{s}_{e}")
                   for e in range(3)] for s in range(NSTREAMS)]
            sacc = [state.tile([2, NS], FP32, tag=f"s{s}", name=f"s{s}")
                    for s in range(NSTREAMS)]
            y16 = [state.tile([2, NS], FP16, tag=f"y16_{s}", name=f"y16_{s}")
                   for s in range(NSTREAMS)]
            yin = [[state.tile([2, NS], FP16, tag=f"yin{s}_{e}", name=f"yin{s}_{e}")
                    for e in range(3)] for s in range(NSTREAMS)]

            def mlp_eval(st, rhs16, h1, h2, h3, e):
                """One MLP eval for stream st; returns the [2, NS] L4 psum."""
                # L1: 2 -> 512
                for m in range(4):
                    p = ps[st].tile([128, NS], FP32, tag=f"p{st}", name=f"p{st}")
                    for n in range(2):
                        nc.tensor.matmul(
                            p[:, 512 * n:512 * (n + 1)],
                            w1t[:, 128 * m:128 * (m + 1)],
                            rhs16[:, 512 * n:512 * (n + 1)],
                            start=True, stop=True)
                    nc.scalar.activation(
                        h1[:, NS * m:NS * (m + 1)], p[:], AF.Tanh,
                        bias=b1et[:, b1col[e] + m:b1col[e] + m + 1])
                # L2: 512 -> 512
                for m2 in range(4):
                    p = ps[st].tile([128, NS], FP32, tag=f"p{st}", name=f"p{st}")
                    for k in range(4):
                        for n in range(2):
                            nc.tensor.matmul(
                                p[:, 512 * n:512 * (n + 1)],
                                w2t[:, (k * 4 + m2) * 128:(k * 4 + m2 + 1) * 128],
                                h1[:, NS * k + 512 * n:NS * k + 512 * (n + 1)],
                                start=(k == 0), stop=(k == 3))
                    nc.scalar.activation(
                        h2[:, NS * m2:NS * (m2 + 1)], p[:], AF.Tanh,
                        bias=b2t[:, m2:m2 + 1])
                # L3: 512 -> 256
                for m3 in range(2):
                    p = ps[st].tile([128, NS], FP32, tag=f"p{st}", name=f"p{st}")
                    for k in range(4):
                        for n in range(2):
                            nc.tensor.matmul(
                                p[:, 512 * n:512 * (n + 1)],
                                w3t[:, (k * 2 + m3) * 128:(k * 2 + m3 + 1) * 128],
                                h2[:, NS * k + 512 * n:NS * k + 512 * (n + 1)],
                                start=(k == 0), stop=(k == 3))
                    nc.scalar.activation(
                        h3[:, NS * m3:NS * (m3 + 1)], p[:], AF.Tanh,
                        bias=b3t[:, m3:m3 + 1])
                # L4: 256 -> 2 (raw psum; p_e = k_e - b4, handled via bias folds)
                p4 = ps[st].tile([2, NS], FP32, tag=f"p{st}", name=f"p4_{st}")
                for k in range(2):
                    for n in range(2):
                        nc.tensor.matmul(
                            p4[:, 512 * n:512 * (n + 1)],
                            w4t[:, 2 * k:2 * k + 2],
                            h3[:, NS * k + 512 * n:NS * k + 512 * (n + 1)],
                            start=(k == 0), stop=(k == 1))
                return p4

            def post_eval(st, e, p4):
                """DVE work after eval e of stream st (p4 = raw L4 psum)."""
                cs = (dt2, dt2, dtf)
                if e < 3:
                    # critical path: next eval's input, straight from PSUM
                    nxt = yin[st][e]
                    nc.vector.scalar_tensor_tensor(
                        nxt[:], p4[:], cs[e], y[st][:], ALU.mult, ALU.add)
                    # off critical path: back up p_e for the final combine
                    nc.vector.tensor_copy(ks[st][e][:], p4[:])
                    if e == 2:
                        # combine partials run during eval 4: sacc = k1 + 2*(k2+k3)
                        sa = sacc[st]
                        nc.vector.tensor_tensor(sa[:], ks[st][1][:], ks[st][2][:], ALU.add)
                        nc.vector.scalar_tensor_tensor(
                            sa[:], sa[:], 2.0, ks[st][0][:], ALU.mult, ALU.add)
                    return nxt
                # e == 3: y += dt/6 * (sacc + p4)   (+ dt*b4 fold if nonzero)
                sa = sacc[st]
                nc.vector.tensor_tensor(sa[:], sa[:], p4[:], ALU.add)
                nc.vector.scalar_tensor_tensor(
                    y[st][:], sa[:], dt6, y[st][:], ALU.mult, ALU.add)
                if b4_nonzero:
                    nc.vector.tensor_scalar(
                        y[st][:], y[st][:], b4ct[:, 0:1], None, ALU.add)
                # fp16 rhs for the next step's eval 1
                nc.vector.tensor_copy(y16[st][:], y[st][:])
                return None

            def one_step(chunks, out_idx):
                """Emit one full RK4 step for all streams.

                out_idx: either a dynamic ds() start (loop var expr) or int.
                """
                h1 = [hp.tile([128, 4 * NS], FP16, tag=f"h1_{s}", name=f"h1_{s}")
                      for s in range(NSTREAMS)]
                h2 = [hp.tile([128, 4 * NS], FP16, tag=f"h2_{s}", name=f"h2_{s}")
                      for s in range(NSTREAMS)]
                h3 = [hp.tile([128, 2 * NS], FP16, tag=f"h3_{s}", name=f"h3_{s}")
                      for s in range(NSTREAMS)]
                rhs = [y16[st] for st in range(NSTREAMS)]
                for e in range(4):
                    new_rhs = []
                    for st in range(NSTREAMS):
                        p4 = mlp_eval(st, rhs[st], h1[st], h2[st], h3[st], e)
                        new_rhs.append(post_eval(st, e, p4))
                    rhs = new_rhs
                for st in range(NSTREAMS):
                    nc.sync.dma_start(
                        out=outp[chunks[st]][ds(out_idx, 1), :, :], in_=y[st][:])

            for phase in range(NPHASES):
                chunks = [phase * NSTREAMS + s for s in range(NSTREAMS)]
                for st in range(NSTREAMS):
                    c0 = chunks[st] * NS
                    nc.sync.dma_start(out=y[st][:], in_=y0t[:, c0:c0 + NS])
                    nc.vector.tensor_copy(y16[st][:], y[st][:])

                with tc.For_i(0, S - 1, 2, hint_engines=(mybir.EngineType.PE,)) as i:
                    one_step(chunks, i)
                    one_step(chunks, i + 1)
                # tail step (S is odd)
                one_step(chunks, S - 1)

    nc.compile()
    return nc


def kernel(y0, t_points, W1, b1, W2, b2, W3, b3, W4, b4):
    y0 = np.asarray(y0, dtype=np.float32)
    t_points = np.asarray(t_points, dtype=np.float32)
    W1 = np.asarray(W1, dtype=np.float32)
    W2 = np.asarray(W2, dtype=np.float32)
    W3 = np.asarray(W3, dtype=np.float32)
    W4 = np.asarray(W4, dtype=np.float32)
    b1 = np.asarray(b1, dtype=np.float32)
    b2 = np.asarray(b2, dtype=np.float32)
    b3 = np.asarray(b3, dtype=np.float32)
    b4 = np.asarray(b4, dtype=np.float32)

    dts = (t_points[1:] - t_points[:-1]).astype(np.float64)
    dt = float(np.mean(dts))
    b4_nonzero = bool(np.any(b4 != 0.0))

    nc = _build_program(dt, b4_nonzero)

    # host-side packing into PE-friendly layouts
    w1p = W1.astype(np.float16)                                   # [2, 512]
    w2p = W2.reshape(4, 128, 4, 128).transpose(1, 0, 2, 3).reshape(128, 2048).astype(np.float16)
    w3p = W3.reshape(4, 128, 2, 128).transpose(1, 0, 2, 3).reshape(128, 1024).astype(np.float16)
    w4p = W4.reshape(2, 128, 2).transpose(1, 0, 2).reshape(128, 4).astype(np.float16)

    bb = (b4.astype(np.float64) @ W1.astype(np.float64))          # [512]
    b1_sets = [b1, (b1 + (dt / 2) * bb).astype(np.float32),
               (b1 + dt * bb).astype(np.float32)]
    b1ep = np.concatenate([np.ascontiguousarray(v.reshape(4, 128).T)
                           for v in b1_sets], axis=1)             # [128, 12]
    b1ep = np.ascontiguousarray(b1ep, dtype=np.float32)
    b2p = np.ascontiguousarray(b2.reshape(4, 128).T)
    b3p = np.ascontiguousarray(b3.reshape(2, 128).T)
    b4cp = np.ascontiguousarray((b4 * dt).astype(np.float32).reshape(2, 1))

    in_maps = []
    for c in range(NCORES):
        y0c = np.ascontiguousarray(y0[c * BC:(c + 1) * BC, :].T)  # [2, 8192]
        in_maps.append({
            "y0t": y0c, "w1": w1p, "w2": w2p, "w3": w3p, "w4": w4p,
            "b1e": b1ep, "b2": b2p, "b3": b3p, "b4c": b4cp,
        })

    res = run_bass_kernel_spmd(nc, in_maps, list(range(NCORES)))

    full = np.empty((T, B, 2), dtype=np.float32)
    full[0] = y0
    for c in range(NCORES):
        oc = res.results[c]["out"]                                # [8, 99, 2, 1024]
        for m in range(NCHUNK):
            col0 = c * BC + m * NS
            full[1:, col0:col0 + NS, :] = oc[m].transpose(0, 2, 1)
    return full


# revision 25
# speedup vs baseline: 1.2441x; 1.0912x over previous
"""Neural ODE (RK4 over a 2->512->512->256->2 tanh MLP) on 8 Trainium2 cores.

Strategy: data-parallel over the batch (65536 rows -> 8192/core), with a
feature-on-partition layout so the MLP weights are PE-stationary and the
batch streams through as the matmul free dimension.  Each core runs the
99 sequential RK4 steps in a hardware loop (For_i, 2 steps per body) over
4 independent batch "streams" of 1024 columns, so TensorE always has
another stream's matmuls to run while one stream waits on tanh (ScalarE)
or the RK4 state update (VectorE); 2 phases cover all 8192 rows.

Numerics: fp16 matmul inputs (weights + activations), fp32 PSUM
accumulation, fp32 state/trajectory.  dt is uniform (t_points = arange*0.01)
and is baked into the instruction stream as immediates.

Critical-path notes: the intermediate RK4 states y+c*k are formed by a
single fused DVE op directly from the layer-4 PSUM (the b4 bias term is
folded into the next eval's layer-1 activation bias), k backups for the
final combine are copied out of PSUM off the critical path, and the
combine partial sums run during eval 4's matmuls.
"""

import numpy as np

import concourse.bass as bass
import concourse.mybir as mybir
import concourse.tile as tile
from concourse import bacc
from concourse.bass import ds
from concourse.bass_utils import run_bass_kernel_spmd

FP32 = mybir.dt.float32
FP16 = mybir.dt.float16
AF = mybir.ActivationFunctionType
ALU = mybir.AluOpType

NCORES = 8
B = 65536
BC = B // NCORES          # 8192 rows per core
T = 100
S = T - 1                 # 99 RK4 steps
H = 512
NS = 1024                 # columns per stream
NSTREAMS = 4
NCHUNK = BC // NS         # 8 chunks of 1024 per core
NPHASES = NCHUNK // NSTREAMS
L1_OFF_PE = True          # layer 1 via GpSimd broadcast + DVE MACs instead of PE


def _build_program(dt: float, b4_nonzero: bool):
    nc = bacc.Bacc("TRN2", num_devices=NCORES)

    y0t = nc.declare_dram_parameter("y0t", [2, BC], FP32, isOutput=False)
    w1 = nc.declare_dram_parameter("w1", [2, H], FP16, isOutput=False)
    w2 = nc.declare_dram_parameter("w2", [128, 2048], FP16, isOutput=False)
    w3 = nc.declare_dram_parameter("w3", [128, 1024], FP16, isOutput=False)
    w4 = nc.declare_dram_parameter("w4", [128, 4], FP16, isOutput=False)
    # per-eval layer-1 biases (b4 folded in: b1_e = b1 + c_e * (b4 @ W1))
    b1e = nc.declare_dram_parameter("b1e", [128, 12], FP32, isOutput=False)
    # W1 as per-partition scalars: w1pp[p, 2*m+c] = W1[c, 128*m+p]
    w1pp = nc.declare_dram_parameter("w1pp", [128, 8], FP32, isOutput=False)
    b2 = nc.declare_dram_parameter("b2", [128, 4], FP32, isOutput=False)
    b3 = nc.declare_dram_parameter("b3", [128, 2], FP32, isOutput=False)
    b4c = nc.declare_dram_parameter("b4c", [2, 1], FP32, isOutput=False)  # dt*b4
    outp = nc.declare_dram_parameter("out", [NCHUNK, S, 2, NS], FP32, isOutput=True)

    dt2, dtf, dt6 = float(dt) * 0.5, float(dt), float(dt) / 6.0
    b1col = [0, 4, 4, 8]

    with tile.TileContext(nc) as tc:
        with (
            tc.tile_pool(name="wp", bufs=1) as wp,
            tc.tile_pool(name="state", bufs=1) as state,
            tc.tile_pool(name="hp", bufs=1) as hp,
            tc.tile_pool(name="ps0", bufs=1, space="PSUM") as ps0,
            tc.tile_pool(name="ps1", bufs=1, space="PSUM") as ps1,
            tc.tile_pool(name="ps2", bufs=1, space="PSUM") as ps2,
            tc.tile_pool(name="ps3", bufs=1, space="PSUM") as ps3,
        ):
            ps = [ps0, ps1, ps2, ps3]

            w1t = wp.tile([2, H], FP16, tag="w1")
            w1ppt = wp.tile([128, 8], FP32, tag="w1pp")
            nc.sync.dma_start(out=w1ppt[:], in_=w1pp[:])
            w2t = wp.tile([128, 2048], FP16, tag="w2")
            w3t = wp.tile([128, 1024], FP16, tag="w3")
            w4t = wp.tile([128, 4], FP16, tag="w4")
            b1et = wp.tile([128, 12], FP32, tag="b1e")
            b2t = wp.tile([128, 4], FP32, tag="b2")
            b3t = wp.tile([128, 2], FP32, tag="b3")
            b4ct = wp.tile([2, 1], FP32, tag="b4c")
            for t_, src in ((w1t, w1), (w2t, w2), (w3t, w3), (w4t, w4),
                            (b1et, b1e), (b2t, b2), (b3t, b3), (b4ct, b4c)):
                nc.sync.dma_start(out=t_[:], in_=src[:])

            # persistent per-stream fp32 state, RK4 accumulator, fp16 rhs
            y = [state.tile([2, NS], FP32, tag=f"y{s}", name=f"y{s}")
                 for s in range(NSTREAMS)]
            acc = [state.tile([2, NS], FP32, tag=f"a{s}", name=f"a{s}")
                   for s in range(NSTREAMS)]
            y16 = [state.tile([2, NS], FP16, tag=f"y16_{s}", name=f"y16_{s}")
                   for s in range(NSTREAMS)]

            def mlp_eval(st, rhs16, h1, h2, h3, e, bc, pre):
                """One MLP eval for stream st; returns the [2, NS] L4 psum."""
                # L1: 2 -> 512
                if L1_OFF_PE:
                    # Row 0 broadcasts directly; row 1 is first moved to
                    # partition 0 of a scratch tile via a tiny SBUF->SBUF DMA
                    # (GpSimd broadcast inputs must start at partition 0, and
                    # DVE cannot shift partitions).  Then per-chunk fused MACs
                    # with per-partition W1 scalars on DVE feed the tanh.
                    r1 = hp.tile([1, NS], FP16, tag=f"r1_{st}",
                                 name=f"r1_{st}", bufs=2)
                    nc.sync.dma_start(out=r1[:], in_=rhs16[1:2, :])
                    nc.gpsimd.partition_broadcast(bc[0][:], rhs16[0:1, :])
                    nc.gpsimd.partition_broadcast(bc[1][:], r1[:])
                    for m in range(4):
                        pm = pre[m]
                        nc.vector.tensor_scalar(
                            pm[:], bc[0][:], w1ppt[:, 2 * m:2 * m + 1],
                            None, ALU.mult)
                        nc.vector.scalar_tensor_tensor(
                            pm[:], bc[1][:], w1ppt[:, 2 * m + 1:2 * m + 2], pm[:],
                            ALU.mult, ALU.add)
                        nc.scalar.activation(
                            h1[:, NS * m:NS * (m + 1)], pm[:], AF.Tanh,
                            bias=b1et[:, b1col[e] + m:b1col[e] + m + 1])
                else:
                    for m in range(4):
                        p = ps[st].tile([128, NS], FP32, tag=f"p{st}", name=f"p{st}")
                        for n in range(2):
                            nc.tensor.matmul(
                                p[:, 512 * n:512 * (n + 1)],
                                w1t[:, 128 * m:128 * (m + 1)],
                                rhs16[:, 512 * n:512 * (n + 1)],
                                start=True, stop=True)
                        nc.scalar.activation(
                            h1[:, NS * m:NS * (m + 1)], p[:], AF.Tanh,
                            bias=b1et[:, b1col[e] + m:b1col[e] + m + 1])
                # L2: 512 -> 512
                for m2 in range(4):
                    p = ps[st].tile([128, NS], FP32, tag=f"p{st}", name=f"p{st}")
                    for k in range(4):
                        for n in range(2):
                            nc.tensor.matmul(
                                p[:, 512 * n:512 * (n + 1)],
                                w2t[:, (k * 4 + m2) * 128:(k * 4 + m2 + 1) * 128],
                                h1[:, NS * k + 512 * n:NS * k + 512 * (n + 1)],
                                start=(k == 0), stop=(k == 3))
                    nc.scalar.activation(
                        h2[:, NS * m2:NS * (m2 + 1)], p[:], AF.Tanh,
                        bias=b2t[:, m2:m2 + 1])
                # L3: 512 -> 256
                for m3 in range(2):
                    p = ps[st].tile([128, NS], FP32, tag=f"p{st}", name=f"p{st}")
                    for k in range(4):
                        for n in range(2):
                            nc.tensor.matmul(
                                p[:, 512 * n:512 * (n + 1)],
                                w3t[:, (k * 2 + m3) * 128:(k * 2 + m3 + 1) * 128],
                                h2[:, NS * k + 512 * n:NS * k + 512 * (n + 1)],
                                start=(k == 0), stop=(k == 3))
                    nc.scalar.activation(
                        h3[:, NS * m3:NS * (m3 + 1)], p[:], AF.Tanh,
                        bias=b3t[:, m3:m3 + 1])
                # L4: 256 -> 2 (raw psum; p_e = k_e - b4, handled via bias folds)
                p4 = ps[st].tile([2, NS], FP32, tag=f"p{st}", name=f"p4_{st}")
                for k in range(2):
                    for n in range(2):
                        nc.tensor.matmul(
                            p4[:, 512 * n:512 * (n + 1)],
                            w4t[:, 2 * k:2 * k + 2],
                            h3[:, NS * k + 512 * n:NS * k + 512 * (n + 1)],
                            start=(k == 0), stop=(k == 1))
                return p4

            def post_eval(st, e, p4):
                """DVE work after eval e of stream st (p4 = raw L4 psum).

                Accumulates acc = p1 + 2*p2 + 2*p3 (+ p4 at the end);
                y_next = y + dt/6 * acc + dt*b4.
                """
                cs = (dt2, dt2, dtf)
                a = acc[st]
                if e < 3:
                    # critical path: next eval's input, straight from PSUM
                    nxt = hp.tile([2, NS], FP16, tag=f"yin{st}",
                                  name=f"yin{st}", bufs=2)
                    nc.vector.scalar_tensor_tensor(
                        nxt[:], p4[:], cs[e], y[st][:], ALU.mult, ALU.add)
                    # off critical path: fold p_e into the RK4 accumulator
                    if e == 0:
                        nc.vector.tensor_copy(a[:], p4[:])
                    else:
                        nc.vector.scalar_tensor_tensor(
                            a[:], p4[:], 2.0, a[:], ALU.mult, ALU.add)
                    return nxt
                # e == 3: y += dt/6 * (acc + p4)   (+ dt*b4 fold if nonzero)
                nc.vector.tensor_tensor(a[:], a[:], p4[:], ALU.add)
                nc.vector.scalar_tensor_tensor(
                    y[st][:], a[:], dt6, y[st][:], ALU.mult, ALU.add)
                if b4_nonzero:
                    nc.vector.tensor_scalar(
                        y[st][:], y[st][:], b4ct[:, 0:1], None, ALU.add)
                # fp16 rhs for the next step's eval 1
                nc.vector.tensor_copy(y16[st][:], y[st][:])
                return None

            def one_step(chunks, out_idx):
                """Emit one full RK4 step for all streams.

                out_idx: either a dynamic ds() start (loop var expr) or int.
                """
                h1 = [hp.tile([128, 4 * NS], FP16, tag=f"h1_{s}", name=f"h1_{s}")
                      for s in range(NSTREAMS)]
                h2 = [hp.tile([128, 4 * NS], FP16, tag=f"h2_{s}", name=f"h2_{s}")
                      for s in range(NSTREAMS)]
                h3 = [hp.tile([128, 2 * NS], FP16, tag=f"h3_{s}", name=f"h3_{s}")
                      for s in range(NSTREAMS)]
                rhs = [y16[st] for st in range(NSTREAMS)]
                for e in range(4):
                    new_rhs = []
                    for st in range(NSTREAMS):
                        bc = [hp.tile([128, NS], FP16, tag=f"bc{st}_{c}",
                                      name=f"bc{st}_{c}", bufs=1) for c in range(2)]
                        pre = [hp.tile([128, NS], FP16, tag=f"pre{st}",
                                       name=f"pre{st}", bufs=2) for m in range(4)]
                        p4 = mlp_eval(st, rhs[st], h1[st], h2[st], h3[st], e, bc, pre)
                        new_rhs.append(post_eval(st, e, p4))
                    rhs = new_rhs
                for st in range(NSTREAMS):
                    nc.sync.dma_start(
                        out=outp[chunks[st]][ds(out_idx, 1), :, :], in_=y[st][:])

            for phase in range(NPHASES):
                chunks = [phase * NSTREAMS + s for s in range(NSTREAMS)]
                for st in range(NSTREAMS):
                    c0 = chunks[st] * NS
                    nc.sync.dma_start(out=y[st][:], in_=y0t[:, c0:c0 + NS])
                    nc.vector.tensor_copy(y16[st][:], y[st][:])

                with tc.For_i(0, S - 1, 2, hint_engines=(mybir.EngineType.PE,)) as i:
                    one_step(chunks, i)
                    one_step(chunks, i + 1)
                # tail step (S is odd)
                one_step(chunks, S - 1)

    nc.compile()
    return nc


def kernel(y0, t_points, W1, b1, W2, b2, W3, b3, W4, b4):
    y0 = np.asarray(y0, dtype=np.float32)
    t_points = np.asarray(t_points, dtype=np.float32)
    W1 = np.asarray(W1, dtype=np.float32)
    W2 = np.asarray(W2, dtype=np.float32)
    W3 = np.asarray(W3, dtype=np.float32)
    W4 = np.asarray(W4, dtype=np.float32)
    b1 = np.asarray(b1, dtype=np.float32)
    b2 = np.asarray(b2, dtype=np.float32)
    b3 = np.asarray(b3, dtype=np.float32)
    b4 = np.asarray(b4, dtype=np.float32)

    dts = (t_points[1:] - t_points[:-1]).astype(np.float64)
    dt = float(np.mean(dts))
    b4_nonzero = bool(np.any(b4 != 0.0))

    nc = _build_program(dt, b4_nonzero)

    # host-side packing into PE-friendly layouts
    w1p = W1.astype(np.float16)                                   # [2, 512]
    # w1pp[p, 2m+c] = W1[c, 128m+p]
    w1ppk = np.ascontiguousarray(
        W1.T.reshape(4, 128, 2).transpose(1, 0, 2).reshape(128, 8), dtype=np.float32)
    w2p = W2.reshape(4, 128, 4, 128).transpose(1, 0, 2, 3).reshape(128, 2048).astype(np.float16)
    w3p = W3.reshape(4, 128, 2, 128).transpose(1, 0, 2, 3).reshape(128, 1024).astype(np.float16)
    w4p = W4.reshape(2, 128, 2).transpose(1, 0, 2).reshape(128, 4).astype(np.float16)

    bb = (b4.astype(np.float64) @ W1.astype(np.float64))          # [512]
    b1_sets = [b1, (b1 + (dt / 2) * bb).astype(np.float32),
               (b1 + dt * bb).astype(np.float32)]
    b1ep = np.concatenate([np.ascontiguousarray(v.reshape(4, 128).T)
                           for v in b1_sets], axis=1)             # [128, 12]
    b1ep = np.ascontiguousarray(b1ep, dtype=np.float32)
    b2p = np.ascontiguousarray(b2.reshape(4, 128).T)
    b3p = np.ascontiguousarray(b3.reshape(2, 128).T)
    b4cp = np.ascontiguousarray((b4 * dt).astype(np.float32).reshape(2, 1))

    in_maps = []
    for c in range(NCORES):
        y0c = np.ascontiguousarray(y0[c * BC:(c + 1) * BC, :].T)  # [2, 8192]
        in_maps.append({
            "y0t": y0c, "w1": w1p, "w1pp": w1ppk, "w2": w2p, "w3": w3p, "w4": w4p,
            "b1e": b1ep, "b2": b2p, "b3": b3p, "b4c": b4cp,
        })

    res = run_bass_kernel_spmd(nc, in_maps, list(range(NCORES)))

    full = np.empty((T, B, 2), dtype=np.float32)
    full[0] = y0
    for c in range(NCORES):
        oc = res.results[c]["out"]                                # [8, 99, 2, 1024]
        for m in range(NCHUNK):
            col0 = c * BC + m * NS
            full[1:, col0:col0 + NS, :] = oc[m].transpose(0, 2, 1)
    return full


# revision 27
# speedup vs baseline: 1.2904x; 1.0372x over previous
"""Neural ODE (RK4 over a 2->512->512->256->2 tanh MLP) on 8 Trainium2 cores.

Strategy: data-parallel over the batch (65536 rows -> 8192/core), with a
feature-on-partition layout so the MLP weights are PE-stationary and the
batch streams through as the matmul free dimension.  Each core runs the
99 sequential RK4 steps in a hardware loop (For_i, 2 steps per body) over
4 independent batch "streams" of 1024 columns, so TensorE always has
another stream's matmuls to run while one stream waits on tanh (ScalarE)
or the RK4 state update (VectorE); 2 phases cover all 8192 rows.

Numerics: fp16 matmul inputs (weights + activations), fp32 PSUM
accumulation, fp32 state/trajectory.  dt is uniform (t_points = arange*0.01)
and is baked into the instruction stream as immediates.

Critical-path notes: the intermediate RK4 states y+c*k are formed by a
single fused DVE op directly from the layer-4 PSUM (the b4 bias term is
folded into the next eval's layer-1 activation bias), k backups for the
final combine are copied out of PSUM off the critical path, and the
combine partial sums run during eval 4's matmuls.
"""

import numpy as np

import concourse.bass as bass
import concourse.mybir as mybir
import concourse.tile as tile
from concourse import bacc
from concourse.bass import ds
from concourse.bass_utils import run_bass_kernel_spmd

FP32 = mybir.dt.float32
FP16 = mybir.dt.float16
AF = mybir.ActivationFunctionType
ALU = mybir.AluOpType

NCORES = 8
B = 65536
BC = B // NCORES          # 8192 rows per core
T = 100
S = T - 1                 # 99 RK4 steps
H = 512
NS = 1024                 # columns per stream
NSTREAMS = 4
NCHUNK = BC // NS         # 8 chunks of 1024 per core
NPHASES = NCHUNK // NSTREAMS
L1_OFF_PE = True          # layer 1 via GpSimd broadcast + DVE MACs instead of PE


def _build_program(dt: float, b4_nonzero: bool):
    nc = bacc.Bacc("TRN2", num_devices=NCORES)

    y0t = nc.declare_dram_parameter("y0t", [2, BC], FP32, isOutput=False)
    w1 = nc.declare_dram_parameter("w1", [2, H], FP16, isOutput=False)
    w2 = nc.declare_dram_parameter("w2", [128, 2048], FP16, isOutput=False)
    w3 = nc.declare_dram_parameter("w3", [128, 1024], FP16, isOutput=False)
    w4 = nc.declare_dram_parameter("w4", [128, 4], FP16, isOutput=False)
    # per-eval layer-1 biases (b4 folded in: b1_e = b1 + c_e * (b4 @ W1))
    b1e = nc.declare_dram_parameter("b1e", [128, 12], FP32, isOutput=False)
    # W1 as per-partition scalars: w1pp[p, 2*m+c] = W1[c, 128*m+p]
    w1pp = nc.declare_dram_parameter("w1pp", [128, 8], FP32, isOutput=False)
    b2 = nc.declare_dram_parameter("b2", [128, 4], FP32, isOutput=False)
    b3 = nc.declare_dram_parameter("b3", [128, 2], FP32, isOutput=False)
    b4c = nc.declare_dram_parameter("b4c", [2, 1], FP32, isOutput=False)  # dt*b4
    outp = nc.declare_dram_parameter("out", [NCHUNK, S, 2, NS], FP32, isOutput=True)

    dt2, dtf, dt6 = float(dt) * 0.5, float(dt), float(dt) / 6.0
    b1col = [0, 4, 4, 8]

    with tile.TileContext(nc) as tc:
        with (
            tc.tile_pool(name="wp", bufs=1) as wp,
            tc.tile_pool(name="state", bufs=1) as state,
            tc.tile_pool(name="hp", bufs=1) as hp,
            tc.tile_pool(name="ps0", bufs=1, space="PSUM") as ps0,
            tc.tile_pool(name="ps1", bufs=1, space="PSUM") as ps1,
            tc.tile_pool(name="ps2", bufs=1, space="PSUM") as ps2,
            tc.tile_pool(name="ps3", bufs=1, space="PSUM") as ps3,
        ):
            ps = [ps0, ps1, ps2, ps3]

            w1t = wp.tile([2, H], FP16, tag="w1")
            w1ppt = wp.tile([128, 8], FP32, tag="w1pp")
            nc.sync.dma_start(out=w1ppt[:], in_=w1pp[:])
            w2t = wp.tile([128, 2048], FP16, tag="w2")
            w3t = wp.tile([128, 1024], FP16, tag="w3")
            w4t = wp.tile([128, 4], FP16, tag="w4")
            b1et = wp.tile([128, 12], FP32, tag="b1e")
            b2t = wp.tile([128, 4], FP32, tag="b2")
            b3t = wp.tile([128, 2], FP32, tag="b3")
            b4ct = wp.tile([2, 1], FP32, tag="b4c")
            for t_, src in ((w1t, w1), (w2t, w2), (w3t, w3), (w4t, w4),
                            (b1et, b1e), (b2t, b2), (b3t, b3), (b4ct, b4c)):
                nc.sync.dma_start(out=t_[:], in_=src[:])

            # persistent per-stream fp32 state, RK4 accumulator, fp16 rhs
            y = [state.tile([2, NS], FP32, tag=f"y{s}", name=f"y{s}")
                 for s in range(NSTREAMS)]
            acc = [state.tile([2, NS], FP32, tag=f"a{s}", name=f"a{s}")
                   for s in range(NSTREAMS)]
            y16 = [state.tile([2, NS], FP16, tag=f"y16_{s}", name=f"y16_{s}")
                   for s in range(NSTREAMS)]

            def mlp_eval(st, rhs16, h1, h2, h3, e, bc, pre):
                """One MLP eval for stream st; returns the [2, NS] L4 psum."""
                # L1: 2 -> 512
                if L1_OFF_PE:
                    # Row 0 broadcasts directly; row 1 is first moved to
                    # partition 0 of a scratch tile via a tiny SBUF->SBUF DMA
                    # (GpSimd broadcast inputs must start at partition 0, and
                    # DVE cannot shift partitions).  Then per-chunk fused MACs
                    # with per-partition W1 scalars on DVE feed the tanh.
                    r1 = hp.tile([1, NS], FP16, tag=f"r1_{st}",
                                 name=f"r1_{st}", bufs=2)
                    nc.sync.dma_start(out=r1[:], in_=rhs16[1:2, :])
                    nc.gpsimd.partition_broadcast(bc[0][:], rhs16[0:1, :])
                    nc.gpsimd.partition_broadcast(bc[1][:], r1[:])
                    for m in range(4):
                        pm = pre[m]
                        nc.vector.tensor_scalar(
                            pm[:], bc[0][:], w1ppt[:, 2 * m:2 * m + 1],
                            None, ALU.mult)
                        nc.vector.scalar_tensor_tensor(
                            pm[:], bc[1][:], w1ppt[:, 2 * m + 1:2 * m + 2], pm[:],
                            ALU.mult, ALU.add)
                        nc.scalar.activation(
                            h1[:, NS * m:NS * (m + 1)], pm[:], AF.Tanh,
                            bias=b1et[:, b1col[e] + m:b1col[e] + m + 1])
                else:
                    for m in range(4):
                        p = ps[st].tile([128, NS], FP32, tag=f"p{st}", name=f"p{st}")
                        for n in range(2):
                            nc.tensor.matmul(
                                p[:, 512 * n:512 * (n + 1)],
                                w1t[:, 128 * m:128 * (m + 1)],
                                rhs16[:, 512 * n:512 * (n + 1)],
                                start=True, stop=True)
                        nc.scalar.activation(
                            h1[:, NS * m:NS * (m + 1)], p[:], AF.Tanh,
                            bias=b1et[:, b1col[e] + m:b1col[e] + m + 1])
                # L2: 512 -> 512
                for m2 in range(4):
                    p = ps[st].tile([128, NS], FP32, tag=f"p{st}", name=f"p{st}")
                    for k in range(4):
                        for n in range(2):
                            nc.tensor.matmul(
                                p[:, 512 * n:512 * (n + 1)],
                                w2t[:, (k * 4 + m2) * 128:(k * 4 + m2 + 1) * 128],
                                h1[:, NS * k + 512 * n:NS * k + 512 * (n + 1)],
                                start=(k == 0), stop=(k == 3))
                    nc.scalar.activation(
                        h2[:, NS * m2:NS * (m2 + 1)], p[:], AF.Tanh,
                        bias=b2t[:, m2:m2 + 1])
                # L3: 512 -> 256
                for m3 in range(2):
                    p = ps[st].tile([128, NS], FP32, tag=f"p{st}", name=f"p{st}")
                    for k in range(4):
                        for n in range(2):
                            nc.tensor.matmul(
                                p[:, 512 * n:512 * (n + 1)],
                                w3t[:, (k * 2 + m3) * 128:(k * 2 + m3 + 1) * 128],
                                h2[:, NS * k + 512 * n:NS * k + 512 * (n + 1)],
                                start=(k == 0), stop=(k == 3))
                    nc.scalar.activation(
                        h3[:, NS * m3:NS * (m3 + 1)], p[:], AF.Tanh,
                        bias=b3t[:, m3:m3 + 1])
                # L4: 256 -> 2 (raw psum; p_e = k_e - b4, handled via bias folds)
                p4 = ps[st].tile([2, NS], FP32, tag=f"p{st}", name=f"p4_{st}")
                for k in range(2):
                    for n in range(2):
                        nc.tensor.matmul(
                            p4[:, 512 * n:512 * (n + 1)],
                            w4t[:, 2 * k:2 * k + 2],
                            h3[:, NS * k + 512 * n:NS * k + 512 * (n + 1)],
                            start=(k == 0), stop=(k == 1))
                return p4

            def post_eval(st, e, p4):
                """DVE work after eval e of stream st (p4 = raw L4 psum).

                Accumulates acc = p1 + 2*p2 + 2*p3 (+ p4 at the end);
                y_next = y + dt/6 * acc + dt*b4.
                """
                cs = (dt2, dt2, dtf)
                a = acc[st]
                if e < 3:
                    # critical path: next eval's input, straight from PSUM
                    nxt = hp.tile([2, NS], FP16, tag=f"yin{st}",
                                  name=f"yin{st}", bufs=2)
                    nc.vector.scalar_tensor_tensor(
                        nxt[:], p4[:], cs[e], y[st][:], ALU.mult, ALU.add)
                    # off critical path: fold p_e into the RK4 accumulator
                    if e == 0:
                        nc.vector.tensor_copy(a[:], p4[:])
                    else:
                        nc.vector.scalar_tensor_tensor(
                            a[:], p4[:], 2.0, a[:], ALU.mult, ALU.add)
                    return nxt
                # e == 3: y += dt/6 * (acc + p4)   (+ dt*b4 fold if nonzero)
                nc.vector.tensor_tensor(a[:], a[:], p4[:], ALU.add)
                nc.vector.scalar_tensor_tensor(
                    y[st][:], a[:], dt6, y[st][:], ALU.mult, ALU.add)
                if b4_nonzero:
                    nc.vector.tensor_scalar(
                        y[st][:], y[st][:], b4ct[:, 0:1], None, ALU.add)
                # fp16 rhs for the next step's eval 1
                nc.vector.tensor_copy(y16[st][:], y[st][:])
                return None

            def one_step(chunks, out_idx):
                """Emit one full RK4 step for all streams.

                out_idx: either a dynamic ds() start (loop var expr) or int.
                """
                h1 = [hp.tile([128, 4 * NS], FP16, tag=f"h1_{s}", name=f"h1_{s}")
                      for s in range(NSTREAMS)]
                h2 = [hp.tile([128, 4 * NS], FP16, tag=f"h2_{s}", name=f"h2_{s}")
                      for s in range(NSTREAMS)]
                h3 = [hp.tile([128, 2 * NS], FP16, tag=f"h3_{s}", name=f"h3_{s}")
                      for s in range(NSTREAMS)]
                rhs = [y16[st] for st in range(NSTREAMS)]
                for e in range(4):
                    new_rhs = []
                    for st in range(NSTREAMS):
                        bc = [hp.tile([128, NS], FP16, tag=f"bc{st}_{c}",
                                      name=f"bc{st}_{c}", bufs=1) for c in range(2)]
                        pre = [hp.tile([128, NS], FP16, tag=f"pre{st}",
                                       name=f"pre{st}", bufs=2) for m in range(4)]
                        p4 = mlp_eval(st, rhs[st], h1[st], h2[st], h3[st], e, bc, pre)
                        new_rhs.append(post_eval(st, e, p4))
                    rhs = new_rhs
                for st in range(NSTREAMS):
                    nc.sync.dma_start(
                        out=outp[chunks[st]][ds(out_idx, 1), :, :], in_=y[st][:])

            for phase in range(NPHASES):
                chunks = [phase * NSTREAMS + s for s in range(NSTREAMS)]
                for st in range(NSTREAMS):
                    c0 = chunks[st] * NS
                    nc.sync.dma_start(out=y[st][:], in_=y0t[:, c0:c0 + NS])
                    nc.vector.tensor_copy(y16[st][:], y[st][:])

                NB = 4  # steps per loop body
                with tc.For_i(0, S - (S % NB), NB,
                              hint_engines=(mybir.EngineType.PE,)) as i:
                    for j in range(NB):
                        one_step(chunks, i + j)
                # tail steps
                for r in range(S - (S % NB), S):
                    one_step(chunks, r)

    nc.compile()
    return nc


def kernel(y0, t_points, W1, b1, W2, b2, W3, b3, W4, b4):
    y0 = np.asarray(y0, dtype=np.float32)
    t_points = np.asarray(t_points, dtype=np.float32)
    W1 = np.asarray(W1, dtype=np.float32)
    W2 = np.asarray(W2, dtype=np.float32)
    W3 = np.asarray(W3, dtype=np.float32)
    W4 = np.asarray(W4, dtype=np.float32)
    b1 = np.asarray(b1, dtype=np.float32)
    b2 = np.asarray(b2, dtype=np.float32)
    b3 = np.asarray(b3, dtype=np.float32)
    b4 = np.asarray(b4, dtype=np.float32)

    dts = (t_points[1:] - t_points[:-1]).astype(np.float64)
    dt = float(np.mean(dts))
    b4_nonzero = bool(np.any(b4 != 0.0))

    nc = _build_program(dt, b4_nonzero)

    # host-side packing into PE-friendly layouts
    w1p = W1.astype(np.float16)                                   # [2, 512]
    # w1pp[p, 2m+c] = W1[c, 128m+p]
    w1ppk = np.ascontiguousarray(
        W1.T.reshape(4, 128, 2).transpose(1, 0, 2).reshape(128, 8), dtype=np.float32)
    w2p = W2.reshape(4, 128, 4, 128).transpose(1, 0, 2, 3).reshape(128, 2048).astype(np.float16)
    w3p = W3.reshape(4, 128, 2, 128).transpose(1, 0, 2, 3).reshape(128, 1024).astype(np.float16)
    w4p = W4.reshape(2, 128, 2).transpose(1, 0, 2).reshape(128, 4).astype(np.float16)

    bb = (b4.astype(np.float64) @ W1.astype(np.float64))          # [512]
    b1_sets = [b1, (b1 + (dt / 2) * bb).astype(np.float32),
               (b1 + dt * bb).astype(np.float32)]
    b1ep = np.concatenate([np.ascontiguousarray(v.reshape(4, 128).T)
                           for v in b1_sets], axis=1)             # [128, 12]
    b1ep = np.ascontiguousarray(b1ep, dtype=np.float32)
    b2p = np.ascontiguousarray(b2.reshape(4, 128).T)
    b3p = np.ascontiguousarray(b3.reshape(2, 128).T)
    b4cp = np.ascontiguousarray((b4 * dt).astype(np.float32).reshape(2, 1))

    in_maps = []
    for c in range(NCORES):
        y0c = np.ascontiguousarray(y0[c * BC:(c + 1) * BC, :].T)  # [2, 8192]
        in_maps.append({
            "y0t": y0c, "w1": w1p, "w1pp": w1ppk, "w2": w2p, "w3": w3p, "w4": w4p,
            "b1e": b1ep, "b2": b2p, "b3": b3p, "b4c": b4cp,
        })

    res = run_bass_kernel_spmd(nc, in_maps, list(range(NCORES)))

    full = np.empty((T, B, 2), dtype=np.float32)
    full[0] = y0
    for c in range(NCORES):
        oc = res.results[c]["out"]                                # [8, 99, 2, 1024]
        for m in range(NCHUNK):
            col0 = c * BC + m * NS
            full[1:, col0:col0 + NS, :] = oc[m].transpose(0, 2, 1)
    return full


# revision 28
# speedup vs baseline: 1.3454x; 1.0426x over previous
"""Neural ODE (RK4 over a 2->512->512->256->2 tanh MLP) on 8 Trainium2 cores.

Strategy: data-parallel over the batch (65536 rows -> 8192/core), with a
feature-on-partition layout so the MLP weights are PE-stationary and the
batch streams through as the matmul free dimension.  Each core runs the
99 sequential RK4 steps in a hardware loop (For_i, 2 steps per body) over
4 independent batch "streams" of 1024 columns, so TensorE always has
another stream's matmuls to run while one stream waits on tanh (ScalarE)
or the RK4 state update (VectorE); 2 phases cover all 8192 rows.

Numerics: fp16 matmul inputs (weights + activations), fp32 PSUM
accumulation, fp32 state/trajectory.  dt is uniform (t_points = arange*0.01)
and is baked into the instruction stream as immediates.

Critical-path notes: the intermediate RK4 states y+c*k are formed by a
single fused DVE op directly from the layer-4 PSUM (the b4 bias term is
folded into the next eval's layer-1 activation bias), k backups for the
final combine are copied out of PSUM off the critical path, and the
combine partial sums run during eval 4's matmuls.
"""

import numpy as np

import concourse.bass as bass
import concourse.mybir as mybir
import concourse.tile as tile
from concourse import bacc
from concourse.bass import ds
from concourse.bass_utils import run_bass_kernel_spmd

FP32 = mybir.dt.float32
FP16 = mybir.dt.float16
AF = mybir.ActivationFunctionType
ALU = mybir.AluOpType

NCORES = 8
B = 65536
BC = B // NCORES          # 8192 rows per core
T = 100
S = T - 1                 # 99 RK4 steps
H = 512
NS = 1024                 # columns per stream
NSTREAMS = 4
NCHUNK = BC // NS         # 8 chunks of 1024 per core
NPHASES = NCHUNK // NSTREAMS
L1_OFF_PE = True          # layer 1 via GpSimd broadcast + DVE MACs instead of PE


def _build_program(dt: float, b4_nonzero: bool):
    nc = bacc.Bacc("TRN2", num_devices=NCORES)

    y0t = nc.declare_dram_parameter("y0t", [2, BC], FP32, isOutput=False)
    w1 = nc.declare_dram_parameter("w1", [2, H], FP16, isOutput=False)
    w2 = nc.declare_dram_parameter("w2", [128, 2048], FP16, isOutput=False)
    w3 = nc.declare_dram_parameter("w3", [128, 1024], FP16, isOutput=False)
    w4 = nc.declare_dram_parameter("w4", [128, 4], FP16, isOutput=False)
    # per-eval layer-1 biases (b4 folded in: b1_e = b1 + c_e * (b4 @ W1))
    b1e = nc.declare_dram_parameter("b1e", [128, 12], FP32, isOutput=False)
    # W1 as per-partition scalars: w1pp[p, 2*m+c] = W1[c, 128*m+p]
    w1pp = nc.declare_dram_parameter("w1pp", [128, 8], FP32, isOutput=False)
    b2 = nc.declare_dram_parameter("b2", [128, 4], FP32, isOutput=False)
    b3 = nc.declare_dram_parameter("b3", [128, 2], FP32, isOutput=False)
    b4c = nc.declare_dram_parameter("b4c", [2, 1], FP32, isOutput=False)  # dt*b4
    outp = nc.declare_dram_parameter("out", [NCHUNK, S, 2, NS], FP32, isOutput=True)

    dt2, dtf, dt6 = float(dt) * 0.5, float(dt), float(dt) / 6.0
    b1col = [0, 4, 4, 8]

    with tile.TileContext(nc) as tc:
        with (
            tc.tile_pool(name="wp", bufs=1) as wp,
            tc.tile_pool(name="state", bufs=1) as state,
            tc.tile_pool(name="hp", bufs=1) as hp,
            tc.tile_pool(name="ps0", bufs=1, space="PSUM") as ps0,
            tc.tile_pool(name="ps1", bufs=1, space="PSUM") as ps1,
            tc.tile_pool(name="ps2", bufs=1, space="PSUM") as ps2,
            tc.tile_pool(name="ps3", bufs=1, space="PSUM") as ps3,
        ):
            ps = [ps0, ps1, ps2, ps3]

            w1t = wp.tile([2, H], FP16, tag="w1")
            w1ppt = wp.tile([128, 8], FP32, tag="w1pp")
            nc.sync.dma_start(out=w1ppt[:], in_=w1pp[:])
            w2t = wp.tile([128, 2048], FP16, tag="w2")
            w3t = wp.tile([128, 1024], FP16, tag="w3")
            w4t = wp.tile([128, 4], FP16, tag="w4")
            b1et = wp.tile([128, 12], FP32, tag="b1e")
            b2t = wp.tile([128, 4], FP32, tag="b2")
            b3t = wp.tile([128, 2], FP32, tag="b3")
            b4ct = wp.tile([2, 1], FP32, tag="b4c")
            for t_, src in ((w1t, w1), (w2t, w2), (w3t, w3), (w4t, w4),
                            (b1et, b1e), (b2t, b2), (b3t, b3), (b4ct, b4c)):
                nc.sync.dma_start(out=t_[:], in_=src[:])

            # persistent per-stream fp32 state, RK4 accumulator, fp16 rhs
            y = [state.tile([2, NS], FP32, tag=f"y{s}", name=f"y{s}")
                 for s in range(NSTREAMS)]
            acc = [state.tile([2, NS], FP32, tag=f"a{s}", name=f"a{s}")
                   for s in range(NSTREAMS)]
            y16 = [state.tile([2, NS], FP16, tag=f"y16_{s}", name=f"y16_{s}")
                   for s in range(NSTREAMS)]

            def mlp_eval(st, rhs16, h1, h2, h3, e, bc, pre):
                """One MLP eval for stream st; returns the [2, NS] L4 psum."""
                # L1: 2 -> 512
                if L1_OFF_PE:
                    # Row 0 broadcasts directly; row 1 is first moved to
                    # partition 0 of a scratch tile via a tiny SBUF->SBUF DMA
                    # (GpSimd broadcast inputs must start at partition 0, and
                    # DVE cannot shift partitions).  Then per-chunk fused MACs
                    # with per-partition W1 scalars on DVE feed the tanh.
                    r1 = hp.tile([1, NS], FP16, tag=f"r1_{st}",
                                 name=f"r1_{st}", bufs=2)
                    nc.sync.dma_start(out=r1[:], in_=rhs16[1:2, :])
                    nc.gpsimd.partition_broadcast(bc[0][:], rhs16[0:1, :])
                    nc.gpsimd.partition_broadcast(bc[1][:], r1[:])
                    for m in range(4):
                        pm = pre[m]
                        nc.vector.tensor_scalar(
                            pm[:], bc[0][:], w1ppt[:, 2 * m:2 * m + 1],
                            None, ALU.mult)
                        nc.vector.scalar_tensor_tensor(
                            pm[:], bc[1][:], w1ppt[:, 2 * m + 1:2 * m + 2], pm[:],
                            ALU.mult, ALU.add)
                        nc.scalar.activation(
                            h1[:, NS * m:NS * (m + 1)], pm[:], AF.Tanh,
                            bias=b1et[:, b1col[e] + m:b1col[e] + m + 1])
                else:
                    for m in range(4):
                        p = ps[st].tile([128, NS], FP32, tag=f"p{st}", name=f"p{st}")
                        for n in range(2):
                            nc.tensor.matmul(
                                p[:, 512 * n:512 * (n + 1)],
                                w1t[:, 128 * m:128 * (m + 1)],
                                rhs16[:, 512 * n:512 * (n + 1)],
                                start=True, stop=True)
                        nc.scalar.activation(
                            h1[:, NS * m:NS * (m + 1)], p[:], AF.Tanh,
                            bias=b1et[:, b1col[e] + m:b1col[e] + m + 1])
                # L2: 512 -> 512
                for m2 in range(4):
                    p = ps[st].tile([128, NS], FP32, tag=f"p{st}", name=f"p{st}")
                    for k in range(4):
                        for n in range(2):
                            nc.tensor.matmul(
                                p[:, 512 * n:512 * (n + 1)],
                                w2t[:, (k * 4 + m2) * 128:(k * 4 + m2 + 1) * 128],
                                h1[:, NS * k + 512 * n:NS * k + 512 * (n + 1)],
                                start=(k == 0), stop=(k == 3))
                    nc.scalar.activation(
                        h2[:, NS * m2:NS * (m2 + 1)], p[:], AF.Tanh,
                        bias=b2t[:, m2:m2 + 1])
                # L3: 512 -> 256
                for m3 in range(2):
                    p = ps[st].tile([128, NS], FP32, tag=f"p{st}", name=f"p{st}")
                    for k in range(4):
                        for n in range(2):
                            nc.tensor.matmul(
                                p[:, 512 * n:512 * (n + 1)],
                                w3t[:, (k * 2 + m3) * 128:(k * 2 + m3 + 1) * 128],
                                h2[:, NS * k + 512 * n:NS * k + 512 * (n + 1)],
                                start=(k == 0), stop=(k == 3))
                    nc.scalar.activation(
                        h3[:, NS * m3:NS * (m3 + 1)], p[:], AF.Tanh,
                        bias=b3t[:, m3:m3 + 1])
                # L4: 256 -> 2 (raw psum; p_e = k_e - b4, handled via bias folds)
                p4 = ps[st].tile([2, NS], FP32, tag=f"p{st}", name=f"p4_{st}")
                for k in range(2):
                    for n in range(2):
                        nc.tensor.matmul(
                            p4[:, 512 * n:512 * (n + 1)],
                            w4t[:, 2 * k:2 * k + 2],
                            h3[:, NS * k + 512 * n:NS * k + 512 * (n + 1)],
                            start=(k == 0), stop=(k == 1))
                return p4

            def post_eval(st, e, p4):
                """DVE work after eval e of stream st (p4 = raw L4 psum).

                Accumulates acc = p1 + 2*p2 + 2*p3 (+ p4 at the end);
                y_next = y + dt/6 * acc + dt*b4.
                """
                cs = (dt2, dt2, dtf)
                a = acc[st]
                if e < 3:
                    # critical path: next eval's input, straight from PSUM
                    nxt = hp.tile([2, NS], FP16, tag=f"yin{st}",
                                  name=f"yin{st}", bufs=2)
                    nc.vector.scalar_tensor_tensor(
                        nxt[:], p4[:], cs[e], y[st][:], ALU.mult, ALU.add)
                    # off critical path: fold p_e into the RK4 accumulator
                    if e == 0:
                        nc.vector.tensor_copy(a[:], p4[:])
                    else:
                        nc.vector.scalar_tensor_tensor(
                            a[:], p4[:], 2.0, a[:], ALU.mult, ALU.add)
                    return nxt
                # e == 3: y += dt/6 * (acc + p4)   (+ dt*b4 fold if nonzero)
                nc.vector.tensor_tensor(a[:], a[:], p4[:], ALU.add)
                nc.vector.scalar_tensor_tensor(
                    y[st][:], a[:], dt6, y[st][:], ALU.mult, ALU.add)
                if b4_nonzero:
                    nc.vector.tensor_scalar(
                        y[st][:], y[st][:], b4ct[:, 0:1], None, ALU.add)
                # fp16 rhs for the next step's eval 1
                nc.vector.tensor_copy(y16[st][:], y[st][:])
                return None

            def one_step(chunks, out_idx):
                """Emit one full RK4 step for all streams.

                out_idx: either a dynamic ds() start (loop var expr) or int.
                """
                h1 = [hp.tile([128, 4 * NS], FP16, tag=f"h1_{s}", name=f"h1_{s}")
                      for s in range(NSTREAMS)]
                h2 = [hp.tile([128, 4 * NS], FP16, tag=f"h2_{s}", name=f"h2_{s}")
                      for s in range(NSTREAMS)]
                h3 = [hp.tile([128, 2 * NS], FP16, tag=f"h3_{s}", name=f"h3_{s}")
                      for s in range(NSTREAMS)]
                rhs = [y16[st] for st in range(NSTREAMS)]
                for e in range(4):
                    new_rhs = []
                    for st in range(NSTREAMS):
                        bc = [hp.tile([128, NS], FP16, tag=f"bc{st}_{c}",
                                      name=f"bc{st}_{c}", bufs=1) for c in range(2)]
                        pre = [hp.tile([128, NS], FP16, tag=f"pre{st}",
                                       name=f"pre{st}", bufs=2) for m in range(4)]
                        p4 = mlp_eval(st, rhs[st], h1[st], h2[st], h3[st], e, bc, pre)
                        new_rhs.append(post_eval(st, e, p4))
                    rhs = new_rhs
                for st in range(NSTREAMS):
                    nc.sync.dma_start(
                        out=outp[chunks[st]][ds(out_idx, 1), :, :], in_=y[st][:])

            for phase in range(NPHASES):
                chunks = [phase * NSTREAMS + s for s in range(NSTREAMS)]
                for st in range(NSTREAMS):
                    c0 = chunks[st] * NS
                    nc.sync.dma_start(out=y[st][:], in_=y0t[:, c0:c0 + NS])
                    nc.vector.tensor_copy(y16[st][:], y[st][:])

                NB = 8  # steps per loop body
                with tc.For_i(0, S - (S % NB), NB,
                              hint_engines=(mybir.EngineType.PE,)) as i:
                    for j in range(NB):
                        one_step(chunks, i + j)
                # tail steps
                for r in range(S - (S % NB), S):
                    one_step(chunks, r)

    nc.compile()
    return nc


def kernel(y0, t_points, W1, b1, W2, b2, W3, b3, W4, b4):
    y0 = np.asarray(y0, dtype=np.float32)
    t_points = np.asarray(t_points, dtype=np.float32)
    W1 = np.asarray(W1, dtype=np.float32)
    W2 = np.asarray(W2, dtype=np.float32)
    W3 = np.asarray(W3, dtype=np.float32)
    W4 = np.asarray(W4, dtype=np.float32)
    b1 = np.asarray(b1, dtype=np.float32)
    b2 = np.asarray(b2, dtype=np.float32)
    b3 = np.asarray(b3, dtype=np.float32)
    b4 = np.asarray(b4, dtype=np.float32)

    dts = (t_points[1:] - t_points[:-1]).astype(np.float64)
    dt = float(np.mean(dts))
    b4_nonzero = bool(np.any(b4 != 0.0))

    nc = _build_program(dt, b4_nonzero)

    # host-side packing into PE-friendly layouts
    w1p = W1.astype(np.float16)                                   # [2, 512]
    # w1pp[p, 2m+c] = W1[c, 128m+p]
    w1ppk = np.ascontiguousarray(
        W1.T.reshape(4, 128, 2).transpose(1, 0, 2).reshape(128, 8), dtype=np.float32)
    w2p = W2.reshape(4, 128, 4, 128).transpose(1, 0, 2, 3).reshape(128, 2048).astype(np.float16)
    w3p = W3.reshape(4, 128, 2, 128).transpose(1, 0, 2, 3).reshape(128, 1024).astype(np.float16)
    w4p = W4.reshape(2, 128, 2).transpose(1, 0, 2).reshape(128, 4).astype(np.float16)

    bb = (b4.astype(np.float64) @ W1.astype(np.float64))          # [512]
    b1_sets = [b1, (b1 + (dt / 2) * bb).astype(np.float32),
               (b1 + dt * bb).astype(np.float32)]
    b1ep = np.concatenate([np.ascontiguousarray(v.reshape(4, 128).T)
                           for v in b1_sets], axis=1)             # [128, 12]
    b1ep = np.ascontiguousarray(b1ep, dtype=np.float32)
    b2p = np.ascontiguousarray(b2.reshape(4, 128).T)
    b3p = np.ascontiguousarray(b3.reshape(2, 128).T)
    b4cp = np.ascontiguousarray((b4 * dt).astype(np.float32).reshape(2, 1))

    in_maps = []
    for c in range(NCORES):
        y0c = np.ascontiguousarray(y0[c * BC:(c + 1) * BC, :].T)  # [2, 8192]
        in_maps.append({
            "y0t": y0c, "w1": w1p, "w1pp": w1ppk, "w2": w2p, "w3": w3p, "w4": w4p,
            "b1e": b1ep, "b2": b2p, "b3": b3p, "b4c": b4cp,
        })

    res = run_bass_kernel_spmd(nc, in_maps, list(range(NCORES)))

    full = np.empty((T, B, 2), dtype=np.float32)
    full[0] = y0
    for c in range(NCORES):
        oc = res.results[c]["out"]                                # [8, 99, 2, 1024]
        for m in range(NCHUNK):
            col0 = c * BC + m * NS
            full[1:, col0:col0 + NS, :] = oc[m].transpose(0, 2, 1)
    return full
